# revision 1
# baseline (speedup 1.0000x reference)
"""Contextual-attention kernel for Trainium2, batch-parallel over 8 NeuronCores.

Per core (one image, feature [256,64,64], shared mask [128,128]):
  1. fd = nearest-downsampled feature, zero-padded       [256, 34, 34]
  2. RW deconv patch banks prebuilt early (f-dependent only): PE transposes
     of contiguous-staged (u,v) grids of f_pad2 -> RW[cc][q, c]
  3. Gram scores S[q,p] = sum over 9 patch-shift outer products (PE matmuls;
     lhsT from contiguous q-strip staging), scaled by
     rnorm[q] = 1/max(||patch_q||, eps)
  4. fuse conv 1 (diag +-1, row-major) and fuse conv 2 (diag +-1, col-major
     incl. wrap slivers): partition shifts staged via per-chunk SBUF->SBUF
     DMA copies pipelined against DVE adds
  5. mask along q, per-p max via PE transposes + free-dim reduce, replicate
     via 1xK ones matmul, subtract, exp(10*x) on ScalarE -> bf16
  6. denominators via ones matmul over q, reciprocal; final weights into
     zero-padded A_pad [q, 34, 34]
  7. deconv: 512 accumulating matmuls vs prebuilt RW -> out[c, parity
     grids]; *0.25

SBUF: one slot-shared "work" pool (4 x 32KB slots, tag "wk") serves all
large buffers with disjoint lifetimes.
"""
import sys

sys.path.insert(0, "/opt/trn_rl_repo")

import numpy as np

import concourse.bass as bass
import concourse.bacc as bacc_mod
import concourse.mybir as mybir
import concourse.tile as tile
from concourse.masks import make_identity
from concourse.bass_utils import run_bass_kernel_spmd

F32 = mybir.dt.float32
F32R = mybir.dt.float32r
BF16 = mybir.dt.bfloat16
AX = mybir.AxisListType
OP = mybir.AluOpType
ACT = mybir.ActivationFunctionType

N_CORES = 8
C, H, W = 256, 64, 64
SCALE = 10.0
EPS = 1e-4


def build_nc(gram_dt="f32r", dec_dt="bf16", reps=1):
    nc = bacc_mod.Bacc("TRN2", target_bir_lowering=False, debug=False)
    feat = nc.dram_tensor("feature", [C, H, W], F32, kind="ExternalInput")
    mask0 = nc.dram_tensor("mask0", [128, 128], F32, kind="ExternalInput")
    out_d = nc.dram_tensor("out", [C, H, W], F32, kind="ExternalOutput")

    assert dec_dt in ("bf16", "f32")
    ddt = BF16 if dec_dt == "bf16" else F32
    gdt = F32R if gram_dt == "f32r" else F32

    with tile.TileContext(nc) as tc:
        with (
            tc.tile_pool(name="fpool", bufs=1) as fpl,
            tc.tile_pool(name="work", bufs=4) as wk,
            tc.tile_pool(name="wr", bufs=4) as wr,
            tc.tile_pool(name="qbp", bufs=2) as qbp,
            tc.tile_pool(name="sml", bufs=1) as sml,
            tc.tile_pool(name="acc", bufs=4, space="PSUM") as pacc,
            tc.tile_pool(name="ptp", bufs=3, space="PSUM") as ptp,
        ):
            for rep in range(reps):
                _body(nc, tc, fpl, wk, wr, qbp, sml, pacc, ptp,
                      feat, mask0, out_d, gdt, ddt, rep)
    nc.finalize()
    return nc


def _body(nc, tc, fpl, wk, wr, qbp, sml, pacc, ptp, feat, mask0, out_d, gdt, ddt, rep):
    # ---------------- constants ----------------
    ident = sml.tile([128, 128], F32, tag="ident")
    make_identity(nc, ident)
    ones128 = sml.tile([128, 128], F32, tag="ones128")
    nc.any.memset(ones128[:], 1.0)
    ones_bf = sml.tile([128, 128], BF16, tag="ones_bf")
    nc.any.memset(ones_bf[:], 1.0)
    ident_bq = sml.tile([128, 128], BF16, tag="ident_bf")
    nc.vector.tensor_copy(ident_bq[:], ident[:])
    ident_d = ident_bq if ddt == BF16 else ident
    ones1 = sml.tile([1, 128], F32, tag="ones1")
    nc.any.memset(ones1[:], 1.0)
    zb32 = sml.tile([128, 1024], F32, tag="zb32")
    nc.any.memset(zb32[:], 0.0)

    # ---------------- stage 0: loads & padded layouts ----------------
    fp2, fdp = [], []
    for cc in range(2):
        t = wk.tile([128, 68, 68], F32, tag="wk", name=f"fp2_{rep}_{cc}")
        nc.any.memset(t[:], 0.0)
        nc.sync.dma_start(t[:, 1:65, 1:65], feat[cc * 128:(cc + 1) * 128])
        fp2.append(t)
    for cc in range(2):
        t = fpl.tile([128, 34, 34], gdt, tag=f"fdp_{cc}")
        tf = t[:].rearrange("p a b -> p (a b)")
        nc.vector.tensor_copy(tf[:, 0:1024], zb32[:])
        nc.vector.tensor_copy(tf[:, 1024:1156], zb32[:, 0:132])
        nc.vector.tensor_copy(t[:, 1:33, 1:33], fp2[cc][:, 1:65:2, 1:65:2])
        fdp.append(t)

    # ---------------- stage 0b: prebuild deconv RW banks ----------------
    # RW[cc][q-part, qc, u*4+v, c] = f_pad2[c, 2qy+u, 2qx+v] transposed
    RW = []
    for cc in range(2):
        rw = wk.tile([128, 8, 16, 128], ddt, tag="wk", name=f"rw_{rep}_{cc}")
        for u in range(4):
            for v in range(4):
                gb = wr.tile([128, 1024], ddt, tag="w", name=f"gb_{rep}_{cc}_{u}_{v}")
                nc.vector.tensor_copy(
                    gb[:].rearrange("p (a b) -> p a b", a=32),
                    fp2[cc][:, u: u + 64: 2, v: v + 64: 2])
                for qc in range(8):
                    ps = ptp.tile([128, 128], ddt, tag="tp")
                    nc.tensor.transpose(ps[:], gb[:, 128 * qc: 128 * (qc + 1)], ident_d[:])
                    nc.any.tensor_copy(rw[:, qc, 4 * u + v, :], ps[:])
        RW.append(rw)

    # ---------------- stage 1: mask -> mm_q [128, 8] ----------------
    msc = sml.tile([1, 3204], F32, tag="msc")
    for k, (dy, dx) in enumerate(((0, 0), (0, 1), (1, 0), (1, 1))):
        off = 0 if k == 0 else 1024
        dst = msc[:, off:off + 1024].rearrange("o (a b) -> o a b", a=32)
        nc.sync.dma_start(dst, mask0[dy::4, dx::4][None])
        if k > 0:
            nc.vector.tensor_add(msc[:, 0:1024], msc[:, 0:1024],
                                 msc[:, 1024:2048])
    msum = msc[:, 0:1024].rearrange("o (a b) -> o a b", a=32)
    mdp = msc[:, 2048:3204].rearrange("o (a b) -> o a b", a=34)
    mbx = msc[:, 1024:2112].rearrange("o (a b) -> o a b", a=34)
    nc.any.memset(mdp[:], 0.0)
    nc.vector.tensor_scalar(mdp[:, 1:33, 1:33], msum[:], 2.5, None, OP.is_ge)
    nc.vector.tensor_add(mbx[:], mdp[:, :, 0:32], mdp[:, :, 1:33])
    nc.vector.tensor_add(mbx[:], mbx[:], mdp[:, :, 2:34])
    mbox = msc[:, 0:1024].rearrange("o (a b) -> o a b", a=32)
    nc.vector.tensor_add(mbox[:], mbx[:, 0:32, :], mbx[:, 1:33, :])
    nc.vector.tensor_add(mbox[:], mbox[:], mbx[:, 2:34, :])
    mmrow = msc[:, 2112:3136]
    nc.vector.tensor_scalar(mmrow[:].rearrange("o (a b) -> o a b", a=32),
                            mbox[:], 0.0, None, OP.is_equal)
    mm_q = sml.tile([128, 8], F32, tag="mm_q")
    for c8 in range(8):
        nc.sync.dma_start(mm_q[:, c8:c8 + 1], mmrow[:, 128 * c8:128 * (c8 + 1)])

    # ---------------- stage 1b: rnorm_q [128, 8] ----------------
    nsc = sml.tile([128, 2244], F32, tag="nsc")
    ssq = nsc[:, 0:1156].rearrange("p (a b) -> p a b", a=34)
    nbx = nsc[:, 1156:2244].rearrange("p (a b) -> p a b", a=34)
    sq = []
    for cc in range(2):
        t = qbp.tile([128, 1156], F32, tag="qb", name=f"sq_{rep}_{cc}")
        nc.scalar.square(t[:], fdp[cc][:].rearrange("p a b -> p (a b)"))
        sq.append(t)
    for (o, n) in ((0, 512), (512, 512), (1024, 132)):
        ps = pacc.tile([128, 512], F32, tag="acc")
        for cc in range(2):
            nc.tensor.matmul(ps[:, :n], ones128[:], sq[cc][:, o:o + n],
                             start=(cc == 0), stop=(cc == 1))
        nc.vector.tensor_copy(nsc[:, o:o + n], ps[:, :n])
    nc.vector.tensor_add(nbx[:], ssq[:, :, 0:32], ssq[:, :, 1:33])
    nc.vector.tensor_add(nbx[:], nbx[:], ssq[:, :, 2:34])
    n2 = nsc[:, 0:1024].rearrange("p (a b) -> p a b", a=32)
    nc.vector.tensor_add(n2[:], nbx[:, 0:32, :], nbx[:, 1:33, :])
    nc.vector.tensor_add(n2[:], n2[:], nbx[:, 2:34, :])
    nrm = nsc[:, 1156:2180]
    nc.scalar.sqrt(nrm[:], nsc[:, 0:1024])
    nc.vector.tensor_scalar_max(nrm[:], nrm[:], EPS)
    nc.vector.reciprocal(nrm[:], nrm[:])
    rnorm_q = sml.tile([128, 8], F32, tag="rnorm_q")
    for c8 in range(8):
        nc.sync.dma_start(rnorm_q[:, c8:c8 + 1], nrm[0:1, 128 * c8:128 * (c8 + 1)])

    # ---------------- stage 2: Gram -> M0[q, p] ----------------
    # 4 passes of 2 q-tiles; per-pass contiguous q-strip staging (lhsT must
    # be a single free run); rhs stays a strided fdp view.
    M0 = wk.tile([128, 8, 1024], F32, tag="wk", name=f"m0_{rep}")
    shifts = [(i, j) for i in range(3) for j in range(3)]
    for t in range(8):
        qb = qbp.tile([128, 2, 9, 128], gdt, tag="qb", name=f"qb_{rep}_{t}")
        for cc in range(2):
            for s, (i, j) in enumerate(shifts):
                nc.vector.tensor_copy(
                    qb[:, cc, s, :].rearrange("p (a b) -> p a b", a=4),
                    fdp[cc][:, i + 4 * t: i + 4 * t + 4, j:j + 32])
        for h in range(2):
            ps = pacc.tile([128, 512], F32, tag="acc")
            k = 0
            for cc in range(2):
                for s, (i, j) in enumerate(shifts):
                    lhsT = qb[:, cc, s, :]
                    rhs = fdp[cc][:, i + 16 * h: i + 16 * h + 16, j:j + 32]
                    nc.tensor.matmul(ps[:], lhsT, rhs,
                                     start=(k == 0), stop=(k == 17))
                    k += 1
            nc.vector.tensor_scalar_mul(M0[:, t, 512 * h: 512 * (h + 1)],
                                        ps[:], rnorm_q[:, t:t + 1])

    # ---------------- stage 3: fuse1 (diag +-1, row-major), per-chunk ----
    # spX[q, j] = M0[q+1, j+1] (0 outside); smX[q, j] = M0[q-1, j-1]
    M1 = wk.tile([128, 8, 1024], F32, tag="wk", name=f"m1_{rep}")
    for ch in range(8):
        sp = wr.tile([128, 1024], F32, tag="w", name=f"sp_{rep}_{ch}")
        nc.sync.dma_start(sp[0:127, 0:1023], M0[1:128, ch, 1:1024])
        if ch < 7:
            nc.sync.dma_start(sp[127:128, 0:1023], M0[0:1, ch + 1, 1:1024])
        else:
            nc.sync.dma_start(sp[127:128, 0:1023], zb32[0:1, 0:1023])
        sm = wr.tile([128, 1024], F32, tag="w", name=f"sm_{rep}_{ch}")
        nc.sync.dma_start(sm[1:128, 1:1024], M0[0:127, ch, 0:1023])
        if ch > 0:
            nc.sync.dma_start(sm[0:1, 1:1024], M0[127:128, ch - 1, 0:1023])
        else:
            nc.sync.dma_start(sm[0:1, 1:1024], zb32[0:1, 0:1023])
        nc.vector.tensor_add(M1[:, ch, 0:1023], M0[:, ch, 0:1023], sp[:, 0:1023])
        nc.vector.tensor_copy(M1[:, ch, 1023:1024], M0[:, ch, 1023:1024])
        nc.vector.tensor_add(M1[:, ch, 1:1024], M1[:, ch, 1:1024], sm[:, 1:1024])

    # ---------------- stage 4: fuse2 (diag +-1, col-major), per-chunk ----
    M0 = wk.tile([128, 8, 1024], F32, tag="wk", name=f"m0b_{rep}")
    for ch in range(8):
        # spX[q, j] = M1[cm+1(q), cm+1(j)]; smX[q, j] = M1[cm-1(q), cm-1(j)]
        sp = wr.tile([128, 1024], F32, tag="w", name=f"s2p_{rep}_{ch}")
        src_hi = M1[32:128, ch] if ch < 7 else None
        if ch < 7:
            nc.sync.dma_start(sp[0:96, 0:992], M1[32:128, ch, 32:1024])
            nc.sync.dma_start(sp[0:96, 992:1023], M1[32:128, ch, 1:32])
            nc.sync.dma_start(sp[96:128, 0:992], M1[0:32, ch + 1, 32:1024])
            nc.sync.dma_start(sp[96:128, 992:1023], M1[0:32, ch + 1, 1:32])
        else:
            nc.sync.dma_start(sp[0:96, 0:992], M1[32:128, 7, 32:1024])
            nc.sync.dma_start(sp[0:96, 992:1023], M1[32:128, 7, 1:32])
            # q-wrap rows: q=992+qx <- M1[qx+1] (qx<=30), q=1023 zero
            nc.sync.dma_start(sp[96:127, 0:992], M1[1:32, 0, 32:1024])
            nc.sync.dma_start(sp[96:127, 992:1023], M1[1:32, 0, 1:32])
            nc.sync.dma_start(sp[127:128, 0:1023], zb32[0:1, 0:1023])
        sm = wr.tile([128, 1024], F32, tag="w", name=f"s2m_{rep}_{ch}")
        if ch > 0:
            nc.sync.dma_start(sm[32:128, 32:1024], M1[0:96, ch, 0:992])
            nc.sync.dma_start(sm[32:128, 1:32], M1[0:96, ch, 992:1023])
            nc.sync.dma_start(sm[0:32, 32:1024], M1[96:128, ch - 1, 0:992])
            nc.sync.dma_start(sm[0:32, 1:32], M1[96:128, ch - 1, 992:1023])
        else:
            nc.sync.dma_start(sm[32:128, 32:1024], M1[0:96, 0, 0:992])
            nc.sync.dma_start(sm[32:128, 1:32], M1[0:96, 0, 992:1023])
            # q-wrap rows: q=qx (1..31) <- M1[991+qx]; q=0 zero
            nc.sync.dma_start(sm[1:32, 32:1024], M1[96:127, 7, 0:992])
            nc.sync.dma_start(sm[1:32, 1:32], M1[96:127, 7, 992:1023])
            nc.sync.dma_start(sm[0:1, 1:1024], zb32[0:1, 0:1023])
        nc.vector.tensor_add(M0[:, ch, 0:1023], M1[:, ch, 0:1023], sp[:, 0:1023])
        nc.vector.tensor_copy(M0[:, ch, 1023:1024], M1[:, ch, 1023:1024])
        nc.vector.tensor_add(M0[:, ch, 1:1024], M0[:, ch, 1:1024], sm[:, 1:1024])

    # ---------------- stage 5: mask, max, exp ----------------
    for t in range(8):
        nc.vector.tensor_scalar_mul(M0[:, t, :], M0[:, t, :], mm_q[:, t:t + 1])
    mx8 = sml.tile([128, 8, 2], F32, tag="mx8")
    for pt in range(8):
        for g in range(2):
            ps = ptp.tile([128, 512], F32, tag="tp", name=f"tpb_{rep}_{pt}_{g}")
            for t4 in range(4):
                t = 4 * g + t4
                nc.tensor.transpose(ps[:, 128 * t4:128 * (t4 + 1)],
                                    M0[:, t, 128 * pt:128 * (pt + 1)], ident[:])
            nc.vector.reduce_max(mx8[:, pt, g:g + 1], ps[:], axis=AX.X)
    mx_all = sml.tile([128, 8], F32, tag="mx_all")
    for pt in range(8):
        nc.vector.reduce_max(mx_all[:, pt:pt + 1], mx8[:, pt, :], axis=AX.X)
    mxrow = sml.tile([1, 1024], F32, tag="mxrow")
    for c8 in range(8):
        nc.sync.dma_start(mxrow[:, 128 * c8:128 * (c8 + 1)], mx_all[:, c8:c8 + 1])
    E = wk.tile([128, 8, 1024], BF16, tag="wk", name=f"e_{rep}")
    for h in range(2):
        psr = pacc.tile([128, 512], F32, tag="acc")
        nc.tensor.matmul(psr[:], ones1[:], mxrow[:, 512 * h:512 * (h + 1)],
                         start=True, stop=True)
        nc.vector.tensor_tensor(
            M1[:, :, 512 * h:512 * (h + 1)], M0[:, :, 512 * h:512 * (h + 1)],
            psr[:].unsqueeze(1).to_broadcast([128, 8, 512]), OP.subtract)
    for t in range(8):
        nc.scalar.activation(E[:, t, :], M1[:, t, :], ACT.Exp, bias=0.0, scale=SCALE)

    # ---------------- stage 5b: denominators -> rcp ----------------
    rcp = sml.tile([128, 1024], F32, tag="rcp")
    for h in range(2):
        pss = pacc.tile([128, 512], F32, tag="acc")
        for t in range(8):
            nc.tensor.matmul(pss[:], ones_bf[:], E[:, t, 512 * h:512 * (h + 1)],
                             start=(t == 0), stop=(t == 7))
        nc.vector.reciprocal(rcp[:, 512 * h:512 * (h + 1)], pss[:])

    # ---------------- stage 5c: final weights -> A_pad ----------------
    A_pad = wk.tile([128, 8, 34, 34], ddt, tag="wk", name=f"ap_{rep}")
    nc.any.memset(A_pad[:].bitcast(F32) if ddt == F32R else A_pad[:], 0.0)
    for t in range(8):
        nc.vector.scalar_tensor_tensor(
            out=A_pad[:, t, 1:33, 1:33],
            in0=E[:, t, :].rearrange("p (a b) -> p a b", a=32),
            scalar=mm_q[:, t:t + 1],
            in1=rcp[:].rearrange("p (a b) -> p a b", a=32),
            op0=OP.mult, op1=OP.mult)

    # ---------------- stage 6: deconv ----------------
    for cc in range(2):
        out_sb = wk.tile([128, 64, 64], F32, tag="wk", name=f"os_{rep}_{cc}")
        for ry in range(2):
            us = [u for u in range(4) if (u + 1) % 2 == ry]
            accs, cnt = {}, {}
            for rx in range(2):
                for h in range(2):
                    accs[(rx, h)] = pacc.tile([128, 512], F32, tag="acc",
                                              name=f"da_{rep}_{cc}_{ry}_{rx}_{h}")
                    cnt[(rx, h)] = 0
            for qc in range(8):
                for rx in range(2):
                    vs = [v for v in range(4) if (v + 1) % 2 == rx]
                    for h in range(2):
                        for u in us:
                            for v in vs:
                                sy = (ry + 1 - u) // 2
                                sx = (rx + 1 - v) // 2
                                rhs = A_pad[:, qc,
                                            1 + sy + 16 * h: 1 + sy + 16 * h + 16,
                                            1 + sx: 1 + sx + 32]
                                k = cnt[(rx, h)]
                                nc.tensor.matmul(accs[(rx, h)][:],
                                                 RW[cc][:, qc, 4 * u + v, :], rhs,
                                                 start=(k == 0), stop=(k == 31))
                                cnt[(rx, h)] += 1
            for rx in range(2):
                for h in range(2):
                    dst = out_sb[:, 32 * h + ry: 32 * (h + 1): 2, rx::2]
                    nc.scalar.mul(dst, accs[(rx, h)][:], 0.25)
        nc.sync.dma_start(out_d[cc * 128:(cc + 1) * 128], out_sb[:])


_NC_CACHE = {}


def _get_nc(cfg=("f32r", "bf16")):
    if cfg not in _NC_CACHE:
        _NC_CACHE[cfg] = build_nc(*cfg)
    return _NC_CACHE[cfg]


def kernel(feature: np.ndarray, mask: np.ndarray) -> np.ndarray:
    feature = np.ascontiguousarray(np.asarray(feature, dtype=np.float32))
    mask = np.asarray(mask, dtype=np.float32)
    nc = _get_nc()
    m0 = np.ascontiguousarray(mask[0, 0])
    in_maps = [{"feature": np.ascontiguousarray(feature[i]), "mask0": m0}
               for i in range(N_CORES)]
    res = run_bass_kernel_spmd(nc, in_maps, list(range(N_CORES)))
    return np.stack([np.asarray(res.results[i]["out"], dtype=np.float32)
                     for i in range(N_CORES)])

